# revision 1
# baseline (speedup 1.0000x reference)
"""Trainium2 Bass kernel for Conv2DCollapse_w_pillar (pillar scatter -> dense BEV).

Strategy ("one-hot matmul scatter"), data-parallel over batch (1 batch / core):
  - Host: dedup pillar rows per flat cell (last write wins, matching the
    reference), sort by cell, bucket into 256-cell blocks, pad each block to K
    rows.  Features are split exactly into 3 bf16 planes (hi/mid/lo) so that
    hi+mid+lo == f32 value bit-exactly.
  - Device: for each pair of blocks, build a one-hot matrix
    oh[i, j] = (cell_id[i] == j) on DVE (is_equal), then 3 accumulating bf16
    matmuls with a block-diagonal stationary operand scatter+transpose the pair
    into PSUM (128 partitions = 2 blocks x 64 channels).  ACT drains PSUM to
    SBUF, big DMAs write the dense (C, ny*nx) plane.  Every output element is
    written exactly once; empty cells get 0 from all-zero one-hot columns.
"""
import sys
sys.path.insert(0, "/opt/trn_rl_repo")
import numpy as np
import ml_dtypes

BF = ml_dtypes.bfloat16
NCORES = 8
C = 64
NX = 512
NY = 512
NXY = NX * NY
BC = 256                 # cells per block
NBLK = NXY // BC         # 1024 blocks per core
NPAIR = NBLK // 2        # 512 pairs per core
CHUNK_PAIRS = 64         # pairs per feature-DMA chunk
NCHUNK = NPAIR // CHUNK_PAIRS
GRP = 8                  # pairs per PSUM group (4 banks)
NSPLIT = 3               # bf16 splits for exact f32

_cache = {}


def _build_nc(K):
    import concourse.bass as bass
    import concourse.tile as tile
    from concourse import bacc, mybir
    from contextlib import ExitStack

    dt = mybir.dt
    K2 = 2 * K
    nc = bacc.Bacc("TRN2", target_bir_lowering=False, debug=False,
                   num_devices=NCORES)
    fe = [nc.dram_tensor(f"fe{s}", [K, NPAIR, C], dt.bfloat16,
                         kind="ExternalInput").ap() for s in range(NSPLIT)]
    fo = [nc.dram_tensor(f"fo{s}", [K, NPAIR, C], dt.bfloat16,
                         kind="ExternalInput").ap() for s in range(NSPLIT)]
    cells_d = nc.dram_tensor("cells", [128, NPAIR], dt.float32,
                             kind="ExternalInput").ap()
    iota_d = nc.dram_tensor("iota", [128, BC], dt.bfloat16,
                            kind="ExternalInput").ap()
    out_d = nc.dram_tensor("out", [C, NXY], dt.float32,
                           kind="ExternalOutput").ap()

    with tile.TileContext(nc) as tc, ExitStack() as ctx:
        const = ctx.enter_context(tc.tile_pool(name="const", bufs=1))
        lhsp = ctx.enter_context(tc.tile_pool(name="lhs", bufs=1))
        ohp = ctx.enter_context(tc.tile_pool(name="oh", bufs=8))
        outp = ctx.enter_context(tc.tile_pool(name="outb", bufs=2))
        psp = ctx.enter_context(tc.tile_pool(name="ps", bufs=2, space="PSUM"))

        iota_t = const.tile([128, BC], dt.bfloat16)
        cells_t = const.tile([128, NPAIR], dt.float32)
        sink = const.tile([128, 2], dt.float32, tag="sink", name="sink")
        nc.gpsimd.dma_start(iota_t[:], iota_d[:])
        nc.gpsimd.dma_start(cells_t[:], cells_d[:])
        # absorber copies: give DVE's clock each preamble-DMA sem one at a time
        # (hardware allows a single embedded sync-wait per instruction)
        nc.vector.tensor_copy(sink[:, 0:1], cells_t[:, 0:1])
        nc.vector.tensor_copy(sink[:, 1:2], iota_t[:, 0:1])

        # persistent zero-stuffed stationary tiles: 2 chunk bufs x NSPLIT
        lhs = [[lhsp.tile([K2, CHUNK_PAIRS * 128], dt.bfloat16,
                          tag=f"lhs{b}_{s}", name=f"lhs{b}_{s}") for s in range(NSPLIT)]
               for b in range(2)]
        for b in range(2):
            for s in range(NSPLIT):
                nc.vector.memset(lhs[b][s][:], 0.0)
        # preamble PE absorber: one ldweights whose wait covers all memsets
        # (single DVE sem lane), so per-chunk absorbers only wait on DMAs
        nc.tensor.ldweights(lhs[1][NSPLIT - 1][0:K, 0:128])

        for c in range(NCHUNK):
            buf = c % 2
            p0 = c * CHUNK_PAIRS
            for s in range(NSPLIT):
                t = lhs[buf][s]
                # even blocks -> rows 0:K, col range [pair*128, pair*128+64)
                dst_e = t[0:K, :].rearrange("k (p f) -> k p f", f=128)[:, :, 0:C]
                nc.sync.dma_start(dst_e, fe[s][:, p0:p0 + CHUNK_PAIRS, :])
                # odd blocks -> rows K:2K, col range [pair*128+64, pair*128+128)
                dst_o = t[K:K2, :].rearrange("k (p f) -> k p f", f=128)[:, :, C:128]
                nc.sync.dma_start(dst_o, fo[s][:, p0:p0 + CHUNK_PAIRS, :])
            for s in range(NSPLIT):
                # absorber: consume the even-DMA sem so real matmuls only
                # need the odd-DMA sem (1-wait limit per instruction)
                nc.tensor.ldweights(lhs[buf][s][0:K, 0:128])
            for g in range(CHUNK_PAIRS // GRP):
                if g % 2 == 0:
                    outb = outp.tile([128, 2 * GRP * BC], dt.float32)
                ps_t = psp.tile([128, GRP * BC], dt.float32)
                for i in range(GRP):
                    p = p0 + g * GRP + i
                    oh = ohp.tile([K2, BC], dt.bfloat16)
                    nc.vector.tensor_scalar(
                        oh[:], iota_t[0:K2, :], cells_t[0:K2, p:p + 1], None,
                        mybir.AluOpType.is_equal)
                    sl = g * GRP + i
                    for s in range(NSPLIT):
                        nc.tensor.matmul(
                            ps_t[:, i * BC:(i + 1) * BC],
                            lhs[buf][s][0:K2, sl * 128:(sl + 1) * 128],
                            oh[:],
                            start=(s == 0), stop=(s == NSPLIT - 1))
                half = (g % 2) * GRP * BC
                nc.scalar.copy(outb[:, half:half + GRP * BC], ps_t[:])
                if g % 2 == 1:
                    base = (p0 + (g - 1) * GRP) * 2 * BC
                    dst4 = out_d[:, base:base + 2 * GRP * 2 * BC].rearrange(
                        "c (p q r) -> c p q r", p=2 * GRP, q=2, r=BC)
                    src_e = outb[0:C, :].rearrange("c (p r) -> c p r", r=BC)
                    src_o = outb[C:128, :].rearrange("c (p r) -> c p r", r=BC)
                    nc.scalar.dma_start(dst4[:, :, 0, :], src_e)
                    nc.scalar.dma_start(dst4[:, :, 1, :], src_o)
    nc.compile()
    return nc


def _prep_core(pf, cell, src, K):
    """pf: (Nb, C) f32 features for this batch (deduped, sorted by cell);
    cell: (Nb,) int cell ids; src unused (rows already gathered)."""
    n = len(cell)
    block = cell // BC
    local = (cell % BC).astype(np.float32)
    starts = np.searchsorted(block, np.arange(NBLK))
    k = np.arange(n) - starts[block]
    assert k.max(initial=0) < K
    pair = block // 2
    parity = block % 2

    x = pf
    hi = x.astype(BF)
    r1 = x - hi.astype(np.float32)
    mid = r1.astype(BF)
    r2 = r1 - mid.astype(np.float32)
    lo = r2.astype(BF)
    assert np.array_equal(
        hi.astype(np.float32) + mid.astype(np.float32) + lo.astype(np.float32), x)
    splits = (hi, mid, lo)

    ev = parity == 0
    od = ~ev
    fe = [np.zeros((K, NPAIR, C), dtype=BF) for _ in range(NSPLIT)]
    fo = [np.zeros((K, NPAIR, C), dtype=BF) for _ in range(NSPLIT)]
    for s in range(NSPLIT):
        fe[s][k[ev], pair[ev], :] = splits[s][ev]
        fo[s][k[od], pair[od], :] = splits[s][od]
    cells = np.full((128, NPAIR), -1.0, np.float32)
    cells[k[ev], pair[ev]] = local[ev]
    cells[K + k[od], pair[od]] = local[od]
    m = {f"fe{s}": fe[s] for s in range(NSPLIT)}
    m.update({f"fo{s}": fo[s] for s in range(NSPLIT)})
    m["cells"] = cells
    m["iota"] = np.broadcast_to(
        np.arange(BC, dtype=np.float32), (128, BC)).astype(BF).copy()
    return m


def kernel(pillar_features, coords, batch_size, nx, ny, num_bev_features,
           **_ignored):
    from concourse import bass_utils

    pf = np.ascontiguousarray(np.asarray(pillar_features, dtype=np.float32))
    co = np.asarray(coords).astype(np.int64)
    B = int(batch_size)
    nx_i, ny_i, C_i = int(nx), int(ny), int(num_bev_features)
    assert (B, nx_i, ny_i, C_i) == (NCORES, NX, NY, C), "hardcoded shape mismatch"

    key = co[:, 0] * NXY + co[:, 1] + co[:, 2] * NX + co[:, 3]
    # dedup, last occurrence wins (matches reference .at[].set semantics)
    n = len(key)
    u, first_rev = np.unique(key[::-1], return_index=True)
    src = n - 1 - first_rev           # original row index that survives
    # u is sorted by (batch, cell)
    batch = (u // NXY).astype(np.int64)
    cell = (u % NXY).astype(np.int64)
    bstart = np.searchsorted(batch, np.arange(NCORES + 1))

    # K: max rows in any 256-cell block, rounded up (shared by all cores)
    blk_global = u // BC
    occ = np.bincount(blk_global - blk_global.min(initial=0)) if len(u) else [0]
    Kmax = int(np.max(np.bincount(blk_global, minlength=1))) if len(u) else 1
    K = max(8, -(-Kmax // 8) * 8)
    assert K <= 64, f"block occupancy {Kmax} too high for pair kernel"

    if K not in _cache:
        _cache[K] = _build_nc(K)
    nc = _cache[K]

    in_maps = []
    for b in range(NCORES):
        lo_i, hi_i = bstart[b], bstart[b + 1]
        in_maps.append(_prep_core(pf[src[lo_i:hi_i]], cell[lo_i:hi_i],
                                  None, K))

    import os
    trace = bool(os.environ.get("BASS_TRACE"))
    res = bass_utils.run_bass_kernel_spmd(
        nc, in_maps, core_ids=list(range(NCORES)), trace=trace)
    kernel._last_results = res

    out = np.empty((NCORES, C, NY, NX), dtype=np.float32)
    for b in range(NCORES):
        out[b] = res.results[b]["out"].reshape(C, NY, NX)
    return out



# revision 22
# speedup vs baseline: 2.5457x; 2.5457x over previous
"""Trainium2 Bass kernel for Conv2DCollapse_w_pillar (pillar scatter -> dense BEV).

Strategy ("one-hot matmul scatter"), data-parallel over batch (1 batch / core):
  - Host: dedup pillar rows per flat cell (last write wins, matching the
    reference), sort by cell, bucket into 256-cell blocks.  Features are
    rounded to a single bf16 plane (harness tolerance is 2e-2 relative; bf16
    rounding contributes ~2e-3) and packed into the exact SBUF stationary
    image per 64-pair chunk: rows 0:K_c hold even blocks (cols pair*128+0:64),
    rows K_c:2K_c odd blocks (cols pair*128+64:128), zero quadrants included,
    so each chunk loads with ONE full-speed contiguous DMA.  K_c is the
    per-chunk max block occupancy across all 8 cores (SPMD shares the
    program), so padding adapts to the data.
  - Device: for each pair of blocks, build a one-hot matrix
    oh[i, j] = (cell_id[i] == j) on DVE (is_equal), then one bf16 matmul with
    the block-diagonal stationary scatter+transposes the pair into PSUM (128
    partitions = 2 blocks x 64 channels).  ACT and Pool drain PSUM to bf16
    SBUF (5/8 vs 3/8 split balancing their rates), SP issues the big dense
    output DMAs; the host upcasts bf16 -> f32.  Every output element is
    written exactly once; empty cells get 0 from all-zero one-hot columns.
"""
import sys
sys.path.insert(0, "/opt/trn_rl_repo")
import numpy as np
import ml_dtypes

BF = ml_dtypes.bfloat16
NCORES = 8
C = 64
NX = 512
NY = 512
NXY = NX * NY
BC = 256                 # cells per block
NBLK = NXY // BC         # 1024 blocks per core
NPAIR = NBLK // 2        # 512 pairs per core
CHUNK_PAIRS = 64         # pairs per feature-DMA chunk
NCHUNK = NPAIR // CHUNK_PAIRS
GRP = 4                  # pairs per PSUM group (2 banks; 4 bufs -> matmuls
                         # depend on drains 4 groups back, off the chain)
WIN = 16                 # pairs per output window (one outb / 2 output DMAs)
ACT_COLS = 736           # of the GRP*BC=1024 PSUM cols per group, ACT drains
                         # 736 and DVE 288 (only ACT/DVE may read PSUM); 6 of
                         # 16 one-hots per window go to Pool so no engine
                         # exceeds ~80% of the DMA pace
NBUF = 5                 # lhs chunk buffers: feature DMA issues 2 chunks ahead
                         # of compute, and the buffer it overwrites went idle
                         # 2 chunks ago, so the issue's embedded wait is stale

_cache = {}


def _build_nc(Ks):
    import concourse.bass as bass
    import concourse.tile as tile
    from concourse import bacc, mybir
    from contextlib import ExitStack

    dt = mybir.dt
    R = [2 * k for k in Ks]
    offs = np.concatenate([[0], np.cumsum(R)]).tolist()
    W = CHUNK_PAIRS * 128
    nc = bacc.Bacc("TRN2", target_bir_lowering=False, debug=False,
                   num_devices=NCORES)
    feat = nc.dram_tensor("feat", [offs[-1], W], dt.bfloat16,
                          kind="ExternalInput").ap()
    cells_d = nc.dram_tensor("cells", [128, NPAIR], dt.float32,
                             kind="ExternalInput").ap()
    iota_d = nc.dram_tensor("iota", [128, BC], dt.bfloat16,
                            kind="ExternalInput").ap()
    out_d = nc.dram_tensor("out", [C, NXY], dt.bfloat16,
                           kind="ExternalOutput").ap()

    with tile.TileContext(nc) as tc, ExitStack() as ctx:
        const = ctx.enter_context(tc.tile_pool(name="const", bufs=1))
        lhsp = ctx.enter_context(tc.tile_pool(name="lhs", bufs=NBUF))
        ohp = ctx.enter_context(tc.tile_pool(name="oh", bufs=24))
        outp = ctx.enter_context(tc.tile_pool(name="outb", bufs=6))
        psp = ctx.enter_context(tc.tile_pool(name="ps", bufs=4, space="PSUM"))

        iota_t = const.tile([128, BC], dt.bfloat16)
        cells_t = const.tile([128, NPAIR], dt.float32)
        sink = const.tile([128, 2], dt.float32, tag="sink", name="sink")
        nc.gpsimd.dma_start(cells_t[:], cells_d[:])
        nc.gpsimd.dma_start(iota_t[:], iota_d[:])
        # absorber copies: give DVE's clock each preamble-DMA sem one at a time
        # (hardware allows a single embedded sync-wait per instruction)
        nc.vector.tensor_copy(sink[:, 0:1], cells_t[:, 0:1])
        nc.vector.tensor_copy(sink[:, 1:2], iota_t[:, 0:1])

        lhs_t = {}

        def issue_feat(cc):
            t = lhsp.tile([R[cc], W], dt.bfloat16)
            lhs_t[cc] = t
            nc.scalar.dma_start(t[:], feat[offs[cc]:offs[cc + 1], :])

        for cc in range(min(3, NCHUNK)):
            issue_feat(cc)

        for c in range(NCHUNK):
            if c + 3 < NCHUNK:
                issue_feat(c + 3)
            t = lhs_t.pop(c)
            K2c = R[c]
            p0 = c * CHUNK_PAIRS
            # absorber: consume the feature-DMA sem on PE's clock so the real
            # matmuls only embed their one-hot (DVE) sem waits
            nc.tensor.ldweights(t[0:K2c, 0:128])
            gpw = WIN // GRP
            for g in range(CHUNK_PAIRS // GRP):
                if g % gpw == 0:
                    outb = outp.tile([128, WIN * BC], dt.bfloat16)
                ps_t = psp.tile([128, GRP * BC], dt.float32)
                for i in range(GRP):
                    p = p0 + g * GRP + i
                    oh = ohp.tile([K2c, BC], dt.bfloat16)
                    eng = nc.gpsimd if (g * GRP + i) % 8 in (2, 5, 7) else nc.vector
                    eng.tensor_scalar(
                        oh[:], iota_t[0:K2c, :], cells_t[0:K2c, p:p + 1], None,
                        mybir.AluOpType.is_equal)
                    sl = g * GRP + i
                    nc.tensor.matmul(
                        ps_t[:, i * BC:(i + 1) * BC],
                        t[0:K2c, sl * 128:(sl + 1) * 128],
                        oh[:],
                        start=True, stop=True)
                half = (g % gpw) * GRP * BC
                nc.scalar.copy(outb[:, half:half + ACT_COLS],
                               ps_t[:, 0:ACT_COLS])
                nc.vector.tensor_copy(outb[:, half + ACT_COLS:half + GRP * BC],
                                      ps_t[:, ACT_COLS:GRP * BC])
                if g % gpw == gpw - 1:
                    base = (p0 + (g - gpw + 1) * GRP) * 2 * BC
                    dst4 = out_d[:, base:base + WIN * 2 * BC].rearrange(
                        "c (p q r) -> c p q r", p=WIN, q=2, r=BC)
                    src_e = outb[0:C, :].rearrange("c (p r) -> c p r", r=BC)
                    src_o = outb[C:128, :].rearrange("c (p r) -> c p r", r=BC)
                    # issue from SP so the multi-sem wait (ACT+Pool drains)
                    # blocks the idle sync sequencer, not ACT's
                    nc.sync.dma_start(dst4[:, :, 0, :], src_e)
                    nc.sync.dma_start(dst4[:, :, 1, :], src_o)
    nc.compile()
    return nc


def _prep_core(pf, cell, Ks, offs):
    """pf: (Nb, C) f32 features for this batch (deduped, sorted by cell);
    cell: (Nb,) int cell ids."""
    n = len(cell)
    block = cell // BC
    local = (cell % BC).astype(np.float32)
    starts = np.searchsorted(block, np.arange(NBLK))
    k = np.arange(n) - starts[block]
    pair = block // 2
    parity = block % 2
    chunk = pair // CHUNK_PAIRS
    Kc = Ks[chunk]
    assert np.all(k < Kc)

    hi = pf.astype(BF)
    W = CHUNK_PAIRS * 128
    feat = np.zeros((offs[-1], W), dtype=BF)
    row = offs[chunk] + parity * Kc + k
    colb = (pair % CHUNK_PAIRS) * 128 + parity * C
    feat[row[:, None], colb[:, None] + np.arange(C)] = hi

    cells = np.full((128, NPAIR), -1.0, np.float32)
    cells[parity * Kc + k, pair] = local
    m = {"feat": feat, "cells": cells}
    m["iota"] = np.broadcast_to(
        np.arange(BC, dtype=np.float32), (128, BC)).astype(BF).copy()
    return m


def kernel(pillar_features, coords, batch_size, nx, ny, num_bev_features,
           **_ignored):
    from concourse import bass_utils

    pf = np.ascontiguousarray(np.asarray(pillar_features, dtype=np.float32))
    co = np.asarray(coords).astype(np.int64)
    B = int(batch_size)
    nx_i, ny_i, C_i = int(nx), int(ny), int(num_bev_features)
    assert (B, nx_i, ny_i, C_i) == (NCORES, NX, NY, C), "hardcoded shape mismatch"

    key = co[:, 0] * NXY + co[:, 1] + co[:, 2] * NX + co[:, 3]
    # dedup, last occurrence wins (matches reference .at[].set semantics)
    n = len(key)
    u, first_rev = np.unique(key[::-1], return_index=True)
    src = n - 1 - first_rev           # original row index that survives
    # u is sorted by (batch, cell)
    batch = (u // NXY).astype(np.int64)
    cell = (u % NXY).astype(np.int64)
    bstart = np.searchsorted(batch, np.arange(NCORES + 1))

    # per-chunk K = max 256-cell-block occupancy across all cores (SPMD: one
    # program shared by the 8 cores), adapting padding to the data
    po = np.zeros((NCORES, NPAIR), np.int64)
    for b in range(NCORES):
        cb = cell[bstart[b]:bstart[b + 1]]
        occ = np.bincount(cb // BC, minlength=NBLK)
        po[b] = np.maximum(occ[0::2], occ[1::2])
    Ks = po.reshape(NCORES, NCHUNK, CHUNK_PAIRS).max(axis=(0, 2))
    Ks = tuple(int(max(4, k)) for k in Ks)
    assert max(Ks) <= 64, f"block occupancy {max(Ks)} too high for pair kernel"
    offs = np.concatenate([[0], np.cumsum([2 * k for k in Ks])])

    if Ks not in _cache:
        _cache[Ks] = _build_nc(Ks)
    nc = _cache[Ks]

    in_maps = []
    for b in range(NCORES):
        lo_i, hi_i = bstart[b], bstart[b + 1]
        in_maps.append(_prep_core(pf[src[lo_i:hi_i]], cell[lo_i:hi_i],
                                  np.asarray(Ks), offs))

    import os
    trace = bool(os.environ.get("BASS_TRACE"))
    res = bass_utils.run_bass_kernel_spmd(
        nc, in_maps, core_ids=list(range(NCORES)), trace=trace)
    kernel._last_results = res

    out = np.empty((NCORES, C, NY, NX), dtype=np.float32)
    for b in range(NCORES):
        out[b] = res.results[b]["out"].astype(np.float32).reshape(C, NY, NX)
    return out


# revision 42
# speedup vs baseline: 2.6607x; 1.0452x over previous
"""Trainium2 Bass kernel for Conv2DCollapse_w_pillar (pillar scatter -> dense BEV).

Strategy ("one-hot matmul scatter"), data-parallel over batch (1 batch / core):
  - Host: dedup pillar rows per flat cell (last write wins, matching the
    reference), sort by cell, bucket into 256-cell blocks.  Features are
    rounded to a single bf16 plane (harness tolerance is 2e-2 relative; bf16
    rounding contributes ~2e-3) and packed into the exact SBUF stationary
    image per 64-pair chunk: rows 0:K_c hold even blocks (cols pair*128+0:64),
    rows K_c:2K_c odd blocks (cols pair*128+64:128), zero quadrants included,
    so each chunk loads with ONE full-speed contiguous DMA.  K_c is the
    per-chunk max block occupancy across all 8 cores (SPMD shares the
    program), so padding adapts to the data.
  - Device: for each pair of blocks, build a one-hot matrix
    oh[i, j] = (cell_id[i] == j) on DVE (is_equal), then one bf16 matmul with
    the block-diagonal stationary scatter+transposes the pair into PSUM (128
    partitions = 2 blocks x 64 channels).  ACT and Pool drain PSUM to bf16
    SBUF (5/8 vs 3/8 split balancing their rates), SP issues the big dense
    output DMAs; the host upcasts bf16 -> f32.  Every output element is
    written exactly once; empty cells get 0 from all-zero one-hot columns.
"""
import sys
sys.path.insert(0, "/opt/trn_rl_repo")
import numpy as np
import ml_dtypes

BF = ml_dtypes.bfloat16
NCORES = 8
C = 64
NX = 512
NY = 512
NXY = NX * NY
BC = 256                 # cells per block
NBLK = NXY // BC         # 1024 blocks per core
NPAIR = NBLK // 2        # 512 pairs per core
CHUNK_PAIRS = 64         # pairs per feature-DMA chunk
NCHUNK = NPAIR // CHUNK_PAIRS
GRP = 4                  # pairs per PSUM group (2 banks; 4 bufs -> matmuls
                         # depend on drains 4 groups back, off the chain)
WIN = 16                 # pairs per output window (one outb / 2 output DMAs)
ACT_COLS = 704           # drain split across a 4-group window: ACT takes
                         # groups 0,2 fully + 704 cols of group 1; DVE takes
                         # 320 of group 1 + group 3 (only ACT/DVE may read
                         # PSUM). 6 of 16 one-hots per window go to Pool.
                         # Keeps every engine under the 2912ns/window DMA pace
NBUF = 5                 # lhs chunk buffers: feature DMA issues 2 chunks ahead
                         # of compute, and the buffer it overwrites went idle
                         # 2 chunks ago, so the issue's embedded wait is stale

_cache = {}


def _build_nc(Ks):
    import concourse.bass as bass
    import concourse.tile as tile
    from concourse import bacc, mybir
    from contextlib import ExitStack

    dt = mybir.dt
    R = [2 * k for k in Ks]
    offs = np.concatenate([[0], np.cumsum(R)]).tolist()
    W = CHUNK_PAIRS * 128
    nc = bacc.Bacc("TRN2", target_bir_lowering=False, debug=False,
                   num_devices=NCORES)
    feat = nc.dram_tensor("feat", [offs[-1], W], dt.bfloat16,
                          kind="ExternalInput").ap()
    cells_d = nc.dram_tensor("cells", [128, NPAIR], dt.float32,
                             kind="ExternalInput").ap()
    iota_d = nc.dram_tensor("iota", [128, BC], dt.bfloat16,
                            kind="ExternalInput").ap()
    out_d = nc.dram_tensor("out", [C, NXY], dt.bfloat16,
                           kind="ExternalOutput").ap()

    with tile.TileContext(nc) as tc, ExitStack() as ctx:
        const = ctx.enter_context(tc.tile_pool(name="const", bufs=1))
        lhsp = ctx.enter_context(tc.tile_pool(name="lhs", bufs=NBUF))
        ohp = ctx.enter_context(tc.tile_pool(name="oh", bufs=32))
        outp = ctx.enter_context(tc.tile_pool(name="outb", bufs=8))
        psp = ctx.enter_context(tc.tile_pool(name="ps", bufs=4, space="PSUM"))

        cells_t = const.tile([128, NPAIR], dt.float32)
        iota_t = const.tile([128, BC], dt.bfloat16)
        # issue from SP FIRST: ACT's queue is stuck behind its act-table load
        # and SP's later feature issues must not beat these small transfers
        # to the DMA FIFO (one-hots need them)
        nc.sync.dma_start(cells_t[:], cells_d[:])
        nc.sync.dma_start(iota_t[:], iota_d[:])

        lhs_t = {}

        def issue_feat(cc):
            t = lhsp.tile([R[cc], W], dt.bfloat16)
            lhs_t[cc] = t
            if cc == 0:
                # quarter the first chunk's transfer so window 0's matmuls
                # start after ~1us of feature data instead of ~4us
                for q in range(4):
                    nc.scalar.dma_start(
                        t[:, q * (W // 4):(q + 1) * (W // 4)],
                        feat[offs[cc]:offs[cc + 1],
                             q * (W // 4):(q + 1) * (W // 4)])
            else:
                # SP issues later chunks so ACT's sequencer stays free for
                # drains during the pipeline-fill phase
                nc.sync.dma_start(t[:], feat[offs[cc]:offs[cc + 1], :])

        for cc in range(min(3, NCHUNK)):
            issue_feat(cc)

        for c in range(NCHUNK):
            if c + 3 < NCHUNK:
                issue_feat(c + 3)
            t = lhs_t.pop(c)
            K2c = R[c]
            p0 = c * CHUNK_PAIRS
            # absorber: consume the feature-DMA sem on PE's clock so the real
            # matmuls only embed their one-hot (DVE) sem waits
            nc.tensor.ldweights(t[0:K2c, 0:128])
            gpw = WIN // GRP
            for g in range(CHUNK_PAIRS // GRP):
                if g % gpw == 0:
                    outb = outp.tile([128, WIN * BC], dt.bfloat16)
                ps_t = psp.tile([128, GRP * BC], dt.float32)
                # Pool takes extra one-hots in the first chunks (it is idle
                # while the window pipeline fills; DMA paces slower there too)
                pool_oh = (2, 4, 5, 7) if c < 2 else (2, 5, 7)
                for i in range(GRP):
                    p = p0 + g * GRP + i
                    oh = ohp.tile([K2c, BC], dt.bfloat16)
                    eng = nc.gpsimd if (g * GRP + i) % 8 in pool_oh else nc.vector
                    eng.tensor_scalar(
                        oh[:], iota_t[0:K2c, :], cells_t[0:K2c, p:p + 1], None,
                        mybir.AluOpType.is_equal)
                    sl = g * GRP + i
                    nc.tensor.matmul(
                        ps_t[:, i * BC:(i + 1) * BC],
                        t[0:K2c, sl * 128:(sl + 1) * 128],
                        oh[:],
                        start=True, stop=True)
                half = (g % gpw) * GRP * BC
                full = GRP * BC
                if g % 4 in (0, 2):
                    nc.scalar.copy(outb[:, half:half + full], ps_t[:])
                elif g % 4 == 1:
                    nc.scalar.copy(outb[:, half:half + ACT_COLS],
                                   ps_t[:, 0:ACT_COLS])
                    nc.vector.tensor_copy(outb[:, half + ACT_COLS:half + full],
                                          ps_t[:, ACT_COLS:full])
                else:
                    nc.vector.tensor_copy(outb[:, half:half + full], ps_t[:])
                if g % gpw == gpw - 1:
                    base = (p0 + (g - gpw + 1) * GRP) * 2 * BC
                    dst4 = out_d[:, base:base + WIN * 2 * BC].rearrange(
                        "c (p q r) -> c p q r", p=WIN, q=2, r=BC)
                    src_e = outb[0:C, :].rearrange("c (p r) -> c p r", r=BC)
                    src_o = outb[C:128, :].rearrange("c (p r) -> c p r", r=BC)
                    # issue from SP so the multi-sem wait (ACT+Pool drains)
                    # blocks the idle sync sequencer, not ACT's
                    nc.sync.dma_start(dst4[:, :, 0, :], src_e)
                    nc.sync.dma_start(dst4[:, :, 1, :], src_o)
    nc.compile()
    return nc


def _prep_core(pf, cell, Ks, offs):
    """pf: (Nb, C) f32 features for this batch (deduped, sorted by cell);
    cell: (Nb,) int cell ids."""
    n = len(cell)
    block = cell // BC
    local = (cell % BC).astype(np.float32)
    starts = np.searchsorted(block, np.arange(NBLK))
    k = np.arange(n) - starts[block]
    pair = block // 2
    parity = block % 2
    chunk = pair // CHUNK_PAIRS
    Kc = Ks[chunk]
    assert np.all(k < Kc)

    hi = pf.astype(BF)
    W = CHUNK_PAIRS * 128
    feat = np.zeros((offs[-1], W), dtype=BF)
    row = offs[chunk] + parity * Kc + k
    colb = (pair % CHUNK_PAIRS) * 128 + parity * C
    feat[row[:, None], colb[:, None] + np.arange(C)] = hi

    cells = np.full((128, NPAIR), -1.0, np.float32)
    cells[parity * Kc + k, pair] = local
    iota = np.broadcast_to(
        np.arange(BC, dtype=np.float32), (128, BC)).astype(BF).copy()
    return {"feat": feat, "cells": cells, "iota": iota}


def kernel(pillar_features, coords, batch_size, nx, ny, num_bev_features,
           **_ignored):
    from concourse import bass_utils

    pf = np.ascontiguousarray(np.asarray(pillar_features, dtype=np.float32))
    co = np.asarray(coords).astype(np.int64)
    B = int(batch_size)
    nx_i, ny_i, C_i = int(nx), int(ny), int(num_bev_features)
    assert (B, nx_i, ny_i, C_i) == (NCORES, NX, NY, C), "hardcoded shape mismatch"

    key = co[:, 0] * NXY + co[:, 1] + co[:, 2] * NX + co[:, 3]
    # dedup, last occurrence wins (matches reference .at[].set semantics)
    n = len(key)
    u, first_rev = np.unique(key[::-1], return_index=True)
    src = n - 1 - first_rev           # original row index that survives
    # u is sorted by (batch, cell)
    batch = (u // NXY).astype(np.int64)
    cell = (u % NXY).astype(np.int64)
    bstart = np.searchsorted(batch, np.arange(NCORES + 1))

    # per-chunk K = max 256-cell-block occupancy across all cores (SPMD: one
    # program shared by the 8 cores), adapting padding to the data
    po = np.zeros((NCORES, NPAIR), np.int64)
    for b in range(NCORES):
        cb = cell[bstart[b]:bstart[b + 1]]
        occ = np.bincount(cb // BC, minlength=NBLK)
        po[b] = np.maximum(occ[0::2], occ[1::2])
    Ks = po.reshape(NCORES, NCHUNK, CHUNK_PAIRS).max(axis=(0, 2))
    Ks = tuple(int(max(4, k)) for k in Ks)
    assert max(Ks) <= 64, f"block occupancy {max(Ks)} too high for pair kernel"
    offs = np.concatenate([[0], np.cumsum([2 * k for k in Ks])])

    if Ks not in _cache:
        _cache[Ks] = _build_nc(Ks)
    nc = _cache[Ks]

    in_maps = []
    for b in range(NCORES):
        lo_i, hi_i = bstart[b], bstart[b + 1]
        in_maps.append(_prep_core(pf[src[lo_i:hi_i]], cell[lo_i:hi_i],
                                  np.asarray(Ks), offs))

    import os
    trace = bool(os.environ.get("BASS_TRACE"))
    res = bass_utils.run_bass_kernel_spmd(
        nc, in_maps, core_ids=list(range(NCORES)), trace=trace)
    kernel._last_results = res

    out = np.empty((NCORES, C, NY, NX), dtype=np.float32)
    for b in range(NCORES):
        out[b] = res.results[b]["out"].astype(np.float32).reshape(C, NY, NX)
    return out


# revision 49
# speedup vs baseline: 2.6824x; 1.0082x over previous
"""Trainium2 Bass kernel for Conv2DCollapse_w_pillar (pillar scatter -> dense BEV).

Strategy ("one-hot matmul scatter"), data-parallel over batch (1 batch / core):
  - Host: dedup pillar rows per flat cell (last write wins, matching the
    reference), sort by cell, bucket into 256-cell blocks.  Features are
    rounded to a single bf16 plane (harness tolerance is 2e-2 relative; bf16
    rounding contributes ~2e-3) and packed into the exact SBUF stationary
    image per 64-pair chunk: rows 0:K_c hold even blocks (cols pair*128+0:64),
    rows K_c:2K_c odd blocks (cols pair*128+64:128), zero quadrants included,
    so each chunk loads with ONE full-speed contiguous DMA.  K_c is the
    per-chunk max block occupancy across all 8 cores (SPMD shares the
    program), so padding adapts to the data.
  - Device: for each pair of blocks, build a one-hot matrix
    oh[i, j] = (cell_id[i] == j) on DVE (is_equal), then one bf16 matmul with
    the block-diagonal stationary scatter+transposes the pair into PSUM (128
    partitions = 2 blocks x 64 channels).  ACT and Pool drain PSUM to bf16
    SBUF (5/8 vs 3/8 split balancing their rates), SP issues the big dense
    output DMAs; the host upcasts bf16 -> f32.  Every output element is
    written exactly once; empty cells get 0 from all-zero one-hot columns.
"""
import sys
sys.path.insert(0, "/opt/trn_rl_repo")
import numpy as np
import ml_dtypes

BF = ml_dtypes.bfloat16
NCORES = 8
C = 64
NX = 512
NY = 512
NXY = NX * NY
BC = 256                 # cells per block
NBLK = NXY // BC         # 1024 blocks per core
NPAIR = NBLK // 2        # 512 pairs per core
CHUNK_PAIRS = 64         # pairs per feature-DMA chunk
NCHUNK = NPAIR // CHUNK_PAIRS
GRP = 4                  # pairs per PSUM group (2 banks; 4 bufs -> matmuls
                         # depend on drains 4 groups back, off the chain)
WIN = 16                 # pairs per output window (one outb / 2 output DMAs)
ACT_COLS = 704           # drain split across a 4-group window: ACT takes
                         # groups 0,2 fully + 704 cols of group 1; DVE takes
                         # 320 of group 1 + group 3 (only ACT/DVE may read
                         # PSUM). 6 of 16 one-hots per window go to Pool.
                         # Keeps every engine under the 2912ns/window DMA pace
NBUF = 5                 # lhs chunk buffers: feature DMA issues 2 chunks ahead
                         # of compute, and the buffer it overwrites went idle
                         # 2 chunks ago, so the issue's embedded wait is stale

_cache = {}


def _build_nc(Ks, wperm):
    import concourse.bass as bass
    import concourse.tile as tile
    from concourse import bacc, mybir
    from contextlib import ExitStack

    dt = mybir.dt
    R = [2 * k for k in Ks]
    offs = np.concatenate([[0], np.cumsum(R)]).tolist()
    W = CHUNK_PAIRS * 128
    nc = bacc.Bacc("TRN2", target_bir_lowering=False, debug=False,
                   num_devices=NCORES)
    feat = nc.dram_tensor("feat", [offs[-1], W], dt.bfloat16,
                          kind="ExternalInput").ap()
    Rmax = max(R)
    cells_d = nc.dram_tensor("cells", [Rmax, NPAIR], dt.float32,
                             kind="ExternalInput").ap()
    iota_d = nc.dram_tensor("iota", [Rmax, BC], dt.bfloat16,
                            kind="ExternalInput").ap()
    out_d = nc.dram_tensor("out", [C, NXY], dt.bfloat16,
                           kind="ExternalOutput").ap()

    with tile.TileContext(nc) as tc, ExitStack() as ctx:
        const = ctx.enter_context(tc.tile_pool(name="const", bufs=1))
        lhsp = ctx.enter_context(tc.tile_pool(name="lhs", bufs=NBUF))
        ohp = ctx.enter_context(tc.tile_pool(name="oh", bufs=32))
        outp = ctx.enter_context(tc.tile_pool(name="outb", bufs=8))
        psp = ctx.enter_context(tc.tile_pool(name="ps", bufs=4, space="PSUM"))

        cells_t = const.tile([Rmax, NPAIR], dt.float32)
        iota_t = const.tile([Rmax, BC], dt.bfloat16)
        # issue from SP FIRST: ACT's queue is stuck behind its act-table load
        # and SP's later feature issues must not beat these small transfers
        # to the DMA FIFO (one-hots need them)
        nc.sync.dma_start(cells_t[:], cells_d[:])
        nc.sync.dma_start(iota_t[:], iota_d[:])

        lhs_t = {}

        def issue_feat(cc):
            t = lhsp.tile([R[cc], W], dt.bfloat16)
            lhs_t[cc] = t
            if cc == 0:
                # quarter the first chunk's transfer so window 0's matmuls
                # start after ~1us of feature data instead of ~4us
                for q in range(4):
                    nc.scalar.dma_start(
                        t[:, q * (W // 4):(q + 1) * (W // 4)],
                        feat[offs[cc]:offs[cc + 1],
                             q * (W // 4):(q + 1) * (W // 4)])
            else:
                # SP issues later chunks so ACT's sequencer stays free for
                # drains during the pipeline-fill phase
                nc.sync.dma_start(t[:], feat[offs[cc]:offs[cc + 1], :])

        for cc in range(min(3, NCHUNK)):
            issue_feat(cc)

        for c in range(NCHUNK):
            if c + 3 < NCHUNK:
                issue_feat(c + 3)
            t = lhs_t.pop(c)
            K2c = R[c]
            p0 = c * CHUNK_PAIRS
            # absorber: consume the feature-DMA sem on PE's clock so the real
            # matmuls only embed their one-hot (DVE) sem waits
            nc.tensor.ldweights(t[0:K2c, 0:128])
            gpw = WIN // GRP
            for g in range(CHUNK_PAIRS // GRP):
                if g % gpw == 0:
                    outb = outp.tile([128, WIN * BC], dt.bfloat16)
                ps_t = psp.tile([128, GRP * BC], dt.float32)
                # Pool takes extra one-hots in the first chunks (it is idle
                # while the window pipeline fills; DMA paces slower there too)
                pool_oh = (2, 4, 5, 7) if c < 2 else (2, 5, 7)
                for i in range(GRP):
                    p = p0 + g * GRP + i
                    oh = ohp.tile([K2c, BC], dt.bfloat16)
                    eng = nc.gpsimd if (g * GRP + i) % 8 in pool_oh else nc.vector
                    eng.tensor_scalar(
                        oh[:], iota_t[0:K2c, :], cells_t[0:K2c, p:p + 1], None,
                        mybir.AluOpType.is_equal)
                    sl = g * GRP + i
                    nc.tensor.matmul(
                        ps_t[:, i * BC:(i + 1) * BC],
                        t[0:K2c, sl * 128:(sl + 1) * 128],
                        oh[:],
                        start=True, stop=True)
                half = (g % gpw) * GRP * BC
                full = GRP * BC
                if g % 4 in (0, 2):
                    nc.scalar.copy(outb[:, half:half + full], ps_t[:])
                elif g % 4 == 1:
                    nc.scalar.copy(outb[:, half:half + ACT_COLS],
                                   ps_t[:, 0:ACT_COLS])
                    nc.vector.tensor_copy(outb[:, half + ACT_COLS:half + full],
                                          ps_t[:, ACT_COLS:full])
                else:
                    nc.vector.tensor_copy(outb[:, half:half + full], ps_t[:])
                if g % gpw == gpw - 1:
                    # windows are processed in occupancy-sorted order (shared
                    # across cores); route each back to its original span
                    slot = (p0 + (g - gpw + 1) * GRP) // WIN
                    base = wperm[slot] * WIN * 2 * BC
                    dst4 = out_d[:, base:base + WIN * 2 * BC].rearrange(
                        "c (p q r) -> c p q r", p=WIN, q=2, r=BC)
                    src_e = outb[0:C, :].rearrange("c (p r) -> c p r", r=BC)
                    src_o = outb[C:128, :].rearrange("c (p r) -> c p r", r=BC)
                    # issue from SP so the multi-sem wait (ACT+Pool drains)
                    # blocks the idle sync sequencer, not ACT's
                    nc.sync.dma_start(dst4[:, :, 0, :], src_e)
                    nc.sync.dma_start(dst4[:, :, 1, :], src_o)
    nc.compile()
    return nc


def _prep_core(pf, cell, Ks, offs, slot_of):
    """pf: (Nb, C) f32 features for this batch (deduped, sorted by cell);
    cell: (Nb,) int cell ids; slot_of[orig_window] -> processing slot."""
    n = len(cell)
    block = cell // BC
    local = (cell % BC).astype(np.float32)
    starts = np.searchsorted(block, np.arange(NBLK))
    k = np.arange(n) - starts[block]
    opair = block // 2
    parity = block % 2
    # remap pairs into occupancy-sorted window slots
    pair = slot_of[opair // WIN] * WIN + opair % WIN
    chunk = pair // CHUNK_PAIRS
    Kc = Ks[chunk]
    assert np.all(k < Kc)

    hi = pf.astype(BF)
    W = CHUNK_PAIRS * 128
    feat = np.zeros((offs[-1], W), dtype=BF)
    row = offs[chunk] + parity * Kc + k
    colb = (pair % CHUNK_PAIRS) * 128 + parity * C
    feat[row[:, None], colb[:, None] + np.arange(C)] = hi

    Rmax = 2 * int(Ks.max())
    cells = np.full((Rmax, NPAIR), -1.0, np.float32)
    cells[parity * Kc + k, pair] = local
    iota = np.broadcast_to(
        np.arange(BC, dtype=np.float32), (Rmax, BC)).astype(BF).copy()
    return {"feat": feat, "cells": cells, "iota": iota}


def kernel(pillar_features, coords, batch_size, nx, ny, num_bev_features,
           **_ignored):
    from concourse import bass_utils

    pf = np.ascontiguousarray(np.asarray(pillar_features, dtype=np.float32))
    co = np.asarray(coords).astype(np.int64)
    B = int(batch_size)
    nx_i, ny_i, C_i = int(nx), int(ny), int(num_bev_features)
    assert (B, nx_i, ny_i, C_i) == (NCORES, NX, NY, C), "hardcoded shape mismatch"

    key = co[:, 0] * NXY + co[:, 1] + co[:, 2] * NX + co[:, 3]
    # dedup, last occurrence wins (matches reference .at[].set semantics)
    n = len(key)
    u, first_rev = np.unique(key[::-1], return_index=True)
    src = n - 1 - first_rev           # original row index that survives
    # u is sorted by (batch, cell)
    batch = (u // NXY).astype(np.int64)
    cell = (u % NXY).astype(np.int64)
    bstart = np.searchsorted(batch, np.arange(NCORES + 1))

    # per-chunk K = max 256-cell-block occupancy across all cores (SPMD: one
    # program shared by the 8 cores).  16-pair windows are sorted by that
    # cross-core occupancy (one shared order) so chunks hold windows of
    # similar K, minimizing padding; output DMAs route each window back to
    # its original span
    po = np.zeros((NCORES, NPAIR), np.int64)
    for b in range(NCORES):
        cb = cell[bstart[b]:bstart[b + 1]]
        occ = np.bincount(cb // BC, minlength=NBLK)
        po[b] = np.maximum(occ[0::2], occ[1::2])
    ccmax = po.max(axis=0)
    wmax = ccmax.reshape(NPAIR // WIN, WIN).max(axis=1)
    wperm = np.argsort(-wmax, kind="stable")      # slot -> original window
    slot_of = np.empty_like(wperm)
    slot_of[wperm] = np.arange(len(wperm))        # original window -> slot
    wpc = CHUNK_PAIRS // WIN                      # windows per chunk
    Ks = wmax[wperm].reshape(NCHUNK, wpc).max(axis=1)
    Ks = tuple(int(max(4, k)) for k in Ks)
    assert max(Ks) <= 64, f"block occupancy {max(Ks)} too high for pair kernel"
    offs = np.concatenate([[0], np.cumsum([2 * k for k in Ks])])

    key_ = (Ks, tuple(int(w) for w in wperm))
    if key_ not in _cache:
        _cache[key_] = _build_nc(Ks, tuple(int(w) for w in wperm))
    nc = _cache[key_]

    in_maps = []
    for b in range(NCORES):
        lo_i, hi_i = bstart[b], bstart[b + 1]
        in_maps.append(_prep_core(pf[src[lo_i:hi_i]], cell[lo_i:hi_i],
                                  np.asarray(Ks), offs, slot_of))

    import os
    trace = bool(os.environ.get("BASS_TRACE"))
    res = bass_utils.run_bass_kernel_spmd(
        nc, in_maps, core_ids=list(range(NCORES)), trace=trace)
    kernel._last_results = res

    out = np.empty((NCORES, C, NY, NX), dtype=np.float32)
    for b in range(NCORES):
        out[b] = res.results[b]["out"].astype(np.float32).reshape(C, NY, NX)
    return out
